# revision 16
# baseline (speedup 1.0000x reference)
# Causal self-attention on 8 NeuronCores (Trainium2, Bass/Tile).
#
# Problem: B=2, T=2048, C=1024, H=16 heads (hd=64).
#   qkv = x @ W_qkv + b_qkv ; per-head causal softmax attention ; y = att_out @ W_proj + b_proj
#
# Sharding: tensor-parallel over heads x data-parallel over batch.
#   core = b*4 + g   (b in {0,1} batch, g in {0..3} head group of 4 heads)
#   Each core: qkv projection for its 4 heads (W_qkv column shard),
#   attention for those heads, then a partial row-shard projection
#   y_partial^T = W_proj[g-rows]^T @ att_out^T.  Host sums the 4 partials
#   per batch and adds b_proj.
#
# On-chip layout is "transposed" (feature-on-partition) throughout so no
# P-matrix transposes are needed beyond the initial xT build:
#   xT [C, T] (PE transpose) -> qT,kT [64, T] per head, v [T, 64] per head
#   with a PREPENDED ones-column so the PV matmul also produces the softmax
#   denominator on PSUM partition 0 (where gpsimd partition_broadcast can
#   read it directly - no staging DMA), S^T [k, q] chunks (exp on ACT),
#   out^T [65, q] accumulated in PSUM rows 1:65, denom in row 0.
#
# The whole kernel is a single software-pipelined stream: the "phase A"
# work (x transposes, qk/v projections) and the output projection are
# chopped into small thunks that are interleaved between the attention
# S -> exp -> PV steps, so the PE stays busy while ACT chews on exp and
# the attention for q-quad qq starts as soon as k/v for t < (qq+1)*512
# exist.  Matmuls use float32r (full PE rate at N>=256) with fp32 acc.

import numpy as np
from collections import deque

T = 2048
C = 1024
HL = 4          # heads per core
HD = 64
CL = HL * HD    # 256 local channels
P = 128

_cache = {}


def _build_nc():
    import concourse.bass as bass
    import concourse.mybir as mybir
    import concourse.tile as tile
    from concourse import bacc
    from concourse.masks import make_identity
    from contextlib import ExitStack

    f32 = mybir.dt.float32
    f32r = mybir.dt.float32r
    EXP = mybir.ActivationFunctionType.Exp

    nc = bacc.Bacc("TRN2", target_bir_lowering=False)
    x_d = nc.declare_dram_parameter("x", [T, C], f32r, isOutput=False)
    wqk_d = nc.declare_dram_parameter("wqk", [C, 2 * CL], f32r, isOutput=False)
    wv_d = nc.declare_dram_parameter("wv", [C, CL], f32r, isOutput=False)
    bqk_d = nc.declare_dram_parameter("bqk", [P, 4], f32, isOutput=False)
    bv_d = nc.declare_dram_parameter("bv", [1, CL], f32, isOutput=False)
    kbias_d = nc.declare_dram_parameter("kbias", [P, 16], f32, isOutput=False)
    wproj_d = nc.declare_dram_parameter("wproj", [CL, C], f32r, isOutput=False)
    yT_d = nc.declare_dram_parameter("yT", [C, T], f32, isOutput=True)

    NT = T // P       # 16 t-tiles of 128
    NCC = C // P      # 8 contraction chunks of 128
    NQ = T // 512     # 4 q-quads of 512

    with tile.TileContext(nc) as tc, ExitStack() as ctx:
        singles = ctx.enter_context(tc.tile_pool(name="singles", bufs=1))

        # persistent SBUF
        qkT = singles.tile([P, 4, T], f32r)         # rows: [q f0,q f1,k f0,k f1]
        vv = singles.tile([P, NT, HL, HD + 1], f32r)  # ones col FIRST, then v
        # attention out^T (c' x t), one tile per q-quad so projection
        # reads never false-depend on a later quad's writes
        ATq = [singles.tile([P, 2, 512], f32r, name=f"AT{i}") for i in range(4)]
        xT = singles.tile([P, NCC, T], f32r)
        wqk_sb = singles.tile([P, NCC, 2 * CL], f32r)
        wv_sb = singles.tile([P, NCC, CL], f32r)
        wproj_sb = singles.tile([P, 2, C], f32r)
        tri01 = singles.tile([P, P], f32)          # lower-tri 1.0 / 0.0
        kbias_sb = singles.tile([P, 16], f32)
        bqk_sb = singles.tile([P, 4], f32)
        bv_sb = singles.tile([P, HL, HD], f32)
        identr = singles.tile([P, P], f32r)

        make_identity(nc, identr.bitcast(f32))
        # tri01[k, q] = 1.0 where q >= k else 0.0
        nc.gpsimd.memset(tri01, 1.0)
        nc.gpsimd.affine_select(
            out=tri01,
            in_=tri01,
            compare_op=mybir.AluOpType.is_ge,
            fill=0.0,
            base=0,
            pattern=[[1, P]],
            channel_multiplier=-1,
        )

        nc.vector.memset(vv[:, :, :, 0].bitcast(f32), 1.0)

        # weights load in chunks as filler thunks (unit_W* below) so the
        # gpsimd queue doesn't serialize 12.6us of weight DMA at startup

        with (
            tc.tile_pool(name="xst", bufs=2) as xst,
            tc.tile_pool(name="pa_ps", bufs=2, space="PSUM") as pa_ps,
            tc.tile_pool(name="ps_s", bufs=2, space="PSUM") as ps_s,
            tc.tile_pool(name="ps_o", bufs=2, space="PSUM") as ps_o,
            tc.tile_pool(name="ptp", bufs=3) as ptp,
            tc.tile_pool(name="ep", bufs=2) as ep,
            tc.tile_pool(name="yst", bufs=3) as yst,
        ):
            # ---- filler thunk machinery ------------------------------------
            # Units of "phase A" / projection work are chopped into small
            # thunks tagged with the quad index that requires their data.
            # flush_stage(s) runs everything needed before quad s; emit_some
            # interleaves thunks into the attention stream as PE filler.
            FQ = deque()

            def emit_some(n):
                for _ in range(n):
                    if not FQ:
                        return
                    FQ.popleft()[1]()

            def flush_stage(s):
                while FQ and FQ[0][0] <= s:
                    FQ.popleft()[1]()

            def flush_all():
                while FQ:
                    FQ.popleft()[1]()

            def unit_Wqk(fi, stage):
                def w():
                    nc.gpsimd.dma_start(
                        out=wqk_sb[:, :, fi * P:(fi + 1) * P],
                        in_=wqk_d[:, fi * P:(fi + 1) * P].rearrange(
                            "(o p) n -> p o n", p=P),
                    )

                FQ.append((stage, w))

            def unit_Wv(half, stage):
                def w():
                    nc.gpsimd.dma_start(
                        out=wv_sb[:, half * 4:(half + 1) * 4, :],
                        in_=wv_d[half * 512:(half + 1) * 512, :].rearrange(
                            "(o p) n -> p o n", p=P),
                    )

                FQ.append((stage, w))

            def unit_Wproj(cc, stage):
                def w():
                    nc.gpsimd.dma_start(
                        out=wproj_sb[:, cc, :],
                        in_=wproj_d[cc * P:(cc + 1) * P, :].rearrange(
                            "(o p) n -> p o n", p=P),
                    )

                FQ.append((stage, w))

            def unit_T(ti, stage):
                # load x tile ti and build xT[:, :, ti*P:(ti+1)*P]
                st = {}

                def t_load():
                    st["xt"] = xst.tile([P, C], f32r, name="xt")
                    # early tiles alternate between the sync and scalar
                    # HWDGE queues so the serial load latency halves
                    deng = nc.scalar if (ti < 4 and ti % 2 == 0) else nc.sync
                    deng.dma_start(out=st["xt"], in_=x_d[ti * P:(ti + 1) * P, :])

                FQ.append((stage, t_load))
                for half in (0, 1):
                    def t_tr(half=half):
                        pt = pa_ps.tile([P, 4, P], f32r, tag="pa")
                        for m in range(4):
                            ci = half * 4 + m
                            nc.tensor.transpose(
                                pt[:, m, :], st["xt"][:, ci * P:(ci + 1) * P], identr
                            )
                        eng = nc.vector if (ti < 4 or (ti + half) % 2 == 0) \
                            else nc.gpsimd
                        eng.tensor_copy(
                            out=xT[:, half * 4:(half + 1) * 4, ti * P:(ti + 1) * P],
                            in_=pt,
                        )

                    FQ.append((stage, t_tr))

            def unit_QK(fi, tj, stage):
                # qkT[:, fi, tj*512:(tj+1)*512] = W chunk^T @ xT + bias
                st = {}

                def q_first():
                    st["pq"] = pa_ps.tile([P, 512], f32, tag="pa", name="pq")
                    for ci in range(2):
                        nc.tensor.matmul(
                            st["pq"],
                            lhsT=wqk_sb[:, ci, fi * P:(fi + 1) * P],
                            rhs=xT[:, ci, tj * 512:(tj + 1) * 512],
                            start=(ci == 0),
                            stop=False,
                        )

                FQ.append((stage, q_first))
                for cb in (2, 4, 6):
                    def q_mid(cb=cb):
                        for ci in range(cb, cb + 2):
                            nc.tensor.matmul(
                                st["pq"],
                                lhsT=wqk_sb[:, ci, fi * P:(fi + 1) * P],
                                rhs=xT[:, ci, tj * 512:(tj + 1) * 512],
                                start=False,
                                stop=(ci == NCC - 1),
                            )

                    FQ.append((stage, q_mid))

                def q_bias():
                    eng = nc.vector if fi % 2 == 0 else nc.gpsimd
                    eng.tensor_scalar_add(
                        out=qkT[:, fi, tj * 512:(tj + 1) * 512],
                        in0=st["pq"],
                        scalar1=bqk_sb[:, fi:fi + 1],
                    )

                FQ.append((stage, q_bias))

            def unit_V(tp, stage):
                # v for t-tiles 2tp, 2tp+1 -> vv[:, ti, :, 1:65]
                st = {}

                def v_first():
                    st["pv"] = pa_ps.tile([P, 2, CL], f32, tag="pa", name="pv")
                    for ci in range(4):
                        nc.tensor.matmul(
                            st["pv"][:, 0, :],
                            lhsT=xT[:, ci, 2 * tp * P:(2 * tp + 1) * P],
                            rhs=wv_sb[:, ci, :],
                            start=(ci == 0),
                            stop=False,
                        )

                FQ.append((stage, v_first))

                def v_second():
                    for ci in range(4, NCC):
                        nc.tensor.matmul(
                            st["pv"][:, 0, :],
                            lhsT=xT[:, ci, 2 * tp * P:(2 * tp + 1) * P],
                            rhs=wv_sb[:, ci, :],
                            start=False,
                            stop=(ci == NCC - 1),
                        )

                FQ.append((stage, v_second))

                def v_third():
                    for ci in range(4):
                        nc.tensor.matmul(
                            st["pv"][:, 1, :],
                            lhsT=xT[:, ci, (2 * tp + 1) * P:(2 * tp + 2) * P],
                            rhs=wv_sb[:, ci, :],
                            start=(ci == 0),
                            stop=False,
                        )

                FQ.append((stage, v_third))

                def v_fourth():
                    for ci in range(4, NCC):
                        nc.tensor.matmul(
                            st["pv"][:, 1, :],
                            lhsT=xT[:, ci, (2 * tp + 1) * P:(2 * tp + 2) * P],
                            rhs=wv_sb[:, ci, :],
                            start=False,
                            stop=(ci == NCC - 1),
                        )

                FQ.append((stage, v_fourth))

                def v_bias(k, ti):
                    eng = nc.vector if k == 0 else nc.gpsimd
                    eng.tensor_add(
                        out=vv[:, ti, :, 1:HD + 1],
                        in0=st["pv"][:, k, :].rearrange("p (h d) -> p h d", h=HL),
                        in1=bv_sb,
                    )
                    # key-padding mask: zero this key's v row AND its ones-col
                    # entry (excludes it from numerator and denominator)
                    eng.tensor_scalar_mul(
                        out=vv[:, ti, :, :],
                        in0=vv[:, ti, :, :],
                        scalar1=kbias_sb[:, ti:ti + 1],
                    )

                FQ.append((stage, lambda: v_bias(0, 2 * tp)))
                FQ.append((stage, lambda: v_bias(1, 2 * tp + 1)))

            def unit_PR(qq, stage, use_act=False):
                # projection for quad qq: yT[:, qq*512:(qq+1)*512]
                for co in range(C // P):
                    def pr(co=co):
                        py = pa_ps.tile([P, 512], f32, tag="pa")
                        for cc in range(2):
                            nc.tensor.matmul(
                                py,
                                lhsT=wproj_sb[:, cc, co * P:(co + 1) * P],
                                rhs=ATq[qq][:, cc, :],
                                start=(cc == 0),
                                stop=(cc == 1),
                            )
                        yt = yst.tile([P, 512], f32)
                        if use_act and co % 3 == 2:
                            nc.scalar.copy(out=yt, in_=py)
                        elif co % 2 == 0:
                            nc.vector.tensor_copy(out=yt, in_=py)
                        else:
                            nc.gpsimd.tensor_copy(out=yt, in_=py)
                        deng = nc.sync if co % 2 == 0 else nc.gpsimd
                        deng.dma_start(
                            out=yT_d[co * P:(co + 1) * P, qq * 512:(qq + 1) * 512],
                            in_=yt,
                        )

                    FQ.append((stage, pr))

            # ---- attention for one (quad, head) ----------------------------
            def attn(qq, h):
                nf = max(1, 3 - qq)   # filler pops per site; deeper early
                bp = (h % 2) * HD
                fo = h // 2
                qTh = qkT[bp:bp + HD, fo, :]
                kTh = qkT[bp:bp + HD, 2 + fo, :]
                po = ps_o.tile([HD + 1, 512], f32)
                qs = qq * 512
                # full (below-diagonal) chunk PAIRS
                for jp in range(2 * qq):
                    j0 = 2 * jp
                    ps2 = ps_s.tile([P, 2, 512], f32, tag="s")
                    for m in range(2):
                        nc.tensor.matmul(
                            ps2[:, m, :],
                            lhsT=kTh[:, (j0 + m) * P:(j0 + m + 1) * P],
                            rhs=qTh[:, qs:qs + 512],
                            start=True,
                            stop=True,
                        )
                    pT2 = ptp.tile([P, 2, 512], f32r, tag="p")
                    nc.scalar.activation(out=pT2, in_=ps2, func=EXP, scale=0.125)
                    emit_some(nf)
                    for m in range(2):
                        nc.tensor.matmul(
                            po,
                            lhsT=vv[:, j0 + m, h, :],
                            rhs=pT2[:, m, :],
                            start=(j0 + m == 0),
                            stop=False,
                        )
                    emit_some(nf)
                # diagonal-region chunks o=0..3 (keys jb..jb+3), packed into
                # two ps_s tiles so the exp batches:
                #   tile A: o0 @ [0:512] (q 0:512), o1 @ [512:896] (q 128:512)
                #   tile B: o2 @ [0:256] (q 256:512), o3 @ [256:512] (q 256:512)
                jb = 4 * qq
                A = ps_s.tile([P, 2, 512], f32, tag="s")
                Af = A.rearrange("p a b -> p (a b)")
                nc.tensor.matmul(
                    A[:, 0, :], lhsT=kTh[:, jb * P:(jb + 1) * P],
                    rhs=qTh[:, qs:qs + 512], start=True, stop=True,
                )
                nc.tensor.matmul(
                    Af[:, 512:896], lhsT=kTh[:, (jb + 1) * P:(jb + 2) * P],
                    rhs=qTh[:, qs + 128:qs + 512], start=True, stop=True,
                )
                pTA = ptp.tile([P, 2, 512], f32r, tag="p")
                pTAf = pTA.rearrange("p a b -> p (a b)")
                nc.scalar.activation(
                    out=pTAf[:, 0:896], in_=Af[:, 0:896], func=EXP, scale=0.125,
                )
                emit_some(nf)
                # causal tri-mask on the first 128 cols of each diag block
                nc.vector.tensor_mul(
                    out=pTA[:, 0, 0:P], in0=pTA[:, 0, 0:P], in1=tri01,
                )
                nc.gpsimd.tensor_mul(
                    out=pTAf[:, 512:512 + P], in0=pTAf[:, 512:512 + P], in1=tri01,
                )
                nc.tensor.matmul(
                    po, lhsT=vv[:, jb, h, :], rhs=pTA[:, 0, :],
                    start=(jb == 0), stop=False,
                )
                nc.tensor.matmul(
                    po[:, 128:], lhsT=vv[:, jb + 1, h, :], rhs=pTAf[:, 512:896],
                    start=False, stop=False,
                )
                B = ps_s.tile([P, 2, 512], f32, tag="s")
                nc.tensor.matmul(
                    B[:, 0, 0:256], lhsT=kTh[:, (jb + 2) * P:(jb + 3) * P],
                    rhs=qTh[:, qs + 256:qs + 512], start=True, stop=True,
                )
                nc.tensor.matmul(
                    B[:, 0, 256:512], lhsT=kTh[:, (jb + 3) * P:(jb + 4) * P],
                    rhs=qTh[:, qs + 256:qs + 512], start=True, stop=True,
                )
                pTB = ptp.tile([P, 2, 512], f32r, tag="p")
                nc.scalar.activation(
                    out=pTB[:, 0, :], in_=B[:, 0, :], func=EXP, scale=0.125,
                )
                emit_some(nf)
                # o3 cols [256:384] (q 256:384 vs keys >= 384) are causally
                # invalid; zero them so the padded-width PV adds nothing
                nc.vector.memset(pTB[:, 0, 256:384].bitcast(f32), 0.0)
                nc.vector.tensor_mul(
                    out=pTB[:, 0, 0:P], in0=pTB[:, 0, 0:P], in1=tri01,
                )
                nc.gpsimd.tensor_mul(
                    out=pTB[:, 0, 384:512], in0=pTB[:, 0, 384:512], in1=tri01,
                )
                nc.tensor.matmul(
                    po[:, 256:], lhsT=vv[:, jb + 2, h, :], rhs=pTB[:, 0, 0:256],
                    start=False, stop=False,
                )
                nc.tensor.matmul(
                    po[:, 256:], lhsT=vv[:, jb + 3, h, :], rhs=pTB[:, 0, 256:512],
                    start=False, stop=True,
                )
                # normalize: rows 1:65 divided by row 0 (the ones-col sum).
                # The denom lives on PSUM partition 0, so partition_broadcast
                # can fan out its reciprocal without any staging DMA.  The
                # mul covers the aligned rows 0:65 (row 0 becomes den/den=1,
                # harmless); the DMA then ships rows 1:65 into AT.
                rcp = ep.tile([1, 512], f32r, tag="rcp")
                with nc.allow_low_precision(
                    reason="f32r reciprocal of softmax denom; 2^-11 rel"
                ):
                    nc.vector.reciprocal(out=rcp, in_=po[0:1, :])
                rb = ep.tile([HD + 1, 512], f32r, tag="rb")
                nc.gpsimd.partition_broadcast(rb, rcp)
                ob = ep.tile([HD + 1, 512], f32r, tag="ob")
                meng = nc.vector if h % 2 == 0 else nc.gpsimd
                meng.tensor_mul(out=ob, in0=po, in1=rb)
                deng = nc.sync if h % 2 == 0 or qq == NQ - 1 else nc.gpsimd
                deng.dma_start(
                    out=ATq[qq][bp:bp + HD, fo, :], in_=ob[1:HD + 1, :],
                )
                emit_some(2 * nf)

            def unit_misc(stage):
                # small params on the gpsimd queue + ACT exp-table preload;
                # emitted mid-prologue so the first x loads go first on
                # their queues
                def m():
                    nc.gpsimd.dma_start(out=kbias_sb, in_=kbias_d[:])
                    nc.gpsimd.dma_start(out=bqk_sb, in_=bqk_d[:])
                    nc.gpsimd.dma_start(
                        out=bv_sb,
                        in_=bv_d[:].rearrange(
                            "o (h d) -> o h d", h=HL).to_broadcast([P, HL, HD]),
                    )
                    warm = ep.tile([1, 8], f32r, tag="rcp", name="warm")
                    nc.scalar.activation(out=warm, in_=tri01[0:1, 0:8], func=EXP)

                FQ.append((stage, m))

            # ---- schedule --------------------------------------------------
            # stage-0 prologue (everything quad 0 needs); weight chunks
            # interleave with the x loads/transposes
            unit_Wqk(0, 0)
            unit_T(0, 0)
            unit_Wqk(2, 0)
            unit_T(1, 0)
            unit_Wv(0, 0)
            unit_T(2, 0)
            unit_Wv(1, 0)
            unit_T(3, 0)
            unit_misc(0)
            unit_QK(0, 0, 0)
            unit_QK(2, 0, 0)
            unit_V(0, 0)
            unit_Wqk(1, 0)
            unit_V(1, 0)
            unit_Wqk(3, 0)
            unit_QK(1, 0, 0)
            unit_QK(3, 0, 0)
            flush_stage(0)

            for qq in range(NQ):
                if qq < NQ - 1:
                    s = qq + 1
                    for ti in range(4 * s, 4 * s + 4):
                        unit_T(ti, s)
                    if s == 1:
                        unit_Wproj(0, s)
                    unit_QK(0, s, s)
                    unit_QK(2, s, s)
                    if s == 1:
                        unit_Wproj(1, s)
                    unit_V(2 * s, s)
                    unit_QK(1, s, s)
                    unit_V(2 * s + 1, s)
                    unit_QK(3, s, s)
                if 1 <= qq <= 2:
                    # projection of the previous quad interleaves as filler
                    # into this quad and must flush before quad qq+1 ends
                    unit_PR(qq - 1, qq + 2)
                for h in range(HL):
                    if qq == NQ - 1 and h == HL - 1:
                        # PR(2) enqueues just before the last head: its
                        # matmuls interleave ahead of the final AT DMA and
                        # keep the PE busy through the normalize chain
                        unit_PR(qq - 1, 99, use_act=True)
                    attn(qq, h)
                if qq < NQ - 1:
                    flush_stage(qq + 1)
            flush_all()
            unit_PR(NQ - 1, 99, use_act=True)
            flush_all()

    return nc


def _get_nc():
    if "nc" not in _cache:
        nc = _build_nc()
        nc.finalize()  # runs the Bacc pass pipeline (reg alloc, wait splitting)
        _cache["nc"] = nc
    return _cache["nc"]


def _make_in_maps(x, attn_mask, W_qkv, b_qkv, W_proj):
    x = np.asarray(x, dtype=np.float32)
    attn_mask = np.asarray(attn_mask)
    W_qkv = np.asarray(W_qkv, dtype=np.float32)
    b_qkv = np.asarray(b_qkv, dtype=np.float32)
    W_proj = np.asarray(W_proj, dtype=np.float32)

    in_maps = []
    for core in range(8):
        b, g = core // 4, core % 4
        s = slice(CL * g, CL * (g + 1))
        wq = W_qkv[:, 0 * C:1 * C][:, s]
        wk = W_qkv[:, 1 * C:2 * C][:, s]
        wv = W_qkv[:, 2 * C:3 * C][:, s]
        bq = b_qkv[0 * C:1 * C][s]
        bk = b_qkv[1 * C:2 * C][s]
        bv = b_qkv[2 * C:3 * C][s]
        bqk = np.concatenate([bq, bk]).reshape(4, P).T  # [128,4], f = fi*128+p
        kbias = (attn_mask[b] != 0).astype(np.float32)  # 0/1 key mask
        in_maps.append({
            "x": np.ascontiguousarray(x[b]),
            "wqk": np.ascontiguousarray(np.concatenate([wq, wk], axis=1)),
            "wv": np.ascontiguousarray(wv),
            "bqk": np.ascontiguousarray(bqk),
            "bv": np.ascontiguousarray(bv.reshape(1, CL)),
            "kbias": np.ascontiguousarray(kbias.reshape(16, P).T),
            "wproj": np.ascontiguousarray(W_proj[s, :]),
        })
    return in_maps


def kernel(x, attn_mask, W_qkv, b_qkv, W_proj, b_proj, _trace=False):
    from concourse.bass_utils import run_bass_kernel_spmd

    nc = _get_nc()
    in_maps = _make_in_maps(x, attn_mask, W_qkv, b_qkv, W_proj)
    res = run_bass_kernel_spmd(nc, in_maps, list(range(8)), trace=_trace)
    outs = res.results

    b_proj = np.asarray(b_proj, dtype=np.float32)
    y = np.empty((2, T, C), dtype=np.float32)
    for b in range(2):
        acc = outs[b * 4]["yT"].T.astype(np.float32).copy()
        for g in range(1, 4):
            acc += outs[b * 4 + g]["yT"].T
        y[b] = acc + b_proj
    if _trace:
        return y, res
    return y
